# revision 1
# baseline (speedup 1.0000x reference)
"""DCGRU cell on 8 Trainium2 NeuronCores.

Strategy (data-parallel over batch B=64 -> 8 per core):
  - Sparse supports are densified on host into S^T [2048, 2048] bf16 and
    streamed column-batched from HBM as matmul stationary operands; the
    diffusion spmm runs as dense bf16 matmul (fp32 PSUM accumulate).
  - Activations live node-major ("natural") [n, (b,f)] in bf16; the
    Chebyshev recursion x2 = 2 S x1 - x0 is folded as xs2' = S x1 -
    0.5 x0 with W2' = 2 W2 (host pre-scales the k=2 W rows).
  - The projection contraction (over features f and matrix index m)
    needs feature-major operands, so each diffusion output is
    PE-transposed into bf16 tiles xs^T. W is host-reordered so state
    features contract as K=64 groups (W rows duplicated at partition
    base 64 so lhsT/rhs bases match) and the 5x2 input features as one
    K=10 group gathered into partition-base-aligned packed tiles.
  - Gate output stays feature-major: r is transposed back and multiplied
    into the natural x0 in place (building the candidate input); u and c
    take a DRAM round-trip; the final GRU combine runs in natural layout
    against an fp32 state re-read, and the output is written as
    [n, (b, u)] fp32 which the host untransposes.
"""

import numpy as np

import concourse.bass as bass
from concourse import bacc
import concourse.mybir as mybir
import concourse.tile as tile
from concourse.bass_utils import run_bass_kernel_spmd
from concourse.masks import make_identity

N = 2048            # nodes
B = 64              # global batch
BL = 8              # batch per core
NCORES = 8
D_IN = 2
U = 64              # hidden units
M = 5               # 1 + 2 supports * K
F = D_IN + U        # 66
NB = N // 128       # 16 node blocks
SC = BL * U         # 512 state cols in natural layout
IC = BL * D_IN      # 16 input cols
CW = SC + IC        # 528 total natural cols
PK = M * D_IN       # 10 packed input rows per batch

F32 = mybir.dt.float32
BF16 = mybir.dt.bfloat16


def _build_nc():
    nc = bacc.Bacc(None, target_bir_lowering=False)

    x0d = nc.declare_dram_parameter("x0", [N, CW], BF16, isOutput=False)
    stfd = nc.declare_dram_parameter("statef", [N, SC], F32, isOutput=False)
    satd = nc.declare_dram_parameter("sat", [N, N], BF16, isOutput=False)
    sbtd = nc.declare_dram_parameter("sbt", [N, N], BF16, isOutput=False)
    wgsd = nc.declare_dram_parameter("wgs", [128, M * 128], BF16, isOutput=False)
    wgid = nc.declare_dram_parameter("wgi", [128, 128], BF16, isOutput=False)
    wcsd = nc.declare_dram_parameter("wcs", [128, M * U], BF16, isOutput=False)
    wcid = nc.declare_dram_parameter("wci", [128, U], BF16, isOutput=False)
    bgd = nc.declare_dram_parameter("bg", [128, 1], F32, isOutput=False)
    bcd = nc.declare_dram_parameter("bc", [U, 1], F32, isOutput=False)
    outd = nc.declare_dram_parameter("out", [N, SC], F32, isOutput=True)
    ubufd = nc.dram_tensor("ubuf", [128, BL * 1024], F32)
    cbufd = nc.dram_tensor("cbuf", [128, BL * 1024], F32)

    with tile.TileContext(nc) as tc:
        _emit(nc, tc, x0d, stfd, satd, sbtd, wgsd, wgid, wcsd, wcid, bgd,
              bcd, outd, ubufd, cbufd)
    nc.compile()
    return nc


def _emit(nc, tc, x0d, stfd, satd, sbtd, wgsd, wgid, wcsd, wcid, bgd, bcd,
          outd, ubufd, cbufd):
    from contextlib import ExitStack
    ctx = ExitStack()
    with ctx:
        consts = ctx.enter_context(tc.tile_pool(name="consts", bufs=1))
        nat = ctx.enter_context(tc.tile_pool(name="nat", bufs=1))
        xst = ctx.enter_context(tc.tile_pool(name="xst", bufs=1))
        x2p = ctx.enter_context(tc.tile_pool(name="x2p", bufs=3))
        spool = ctx.enter_context(tc.tile_pool(name="spool", bufs=3))
        small = ctx.enter_context(tc.tile_pool(name="small", bufs=2))
        psum = ctx.enter_context(tc.tile_pool(name="psum", bufs=8, space="PSUM"))

        ident = consts.tile([128, 128], F32)
        make_identity(nc, ident[:])
        identb = consts.tile([128, 128], BF16)
        nc.vector.tensor_copy(identb[:], ident[:])

        wgs = consts.tile([128, M * 128], BF16)
        wgi = consts.tile([128, 128], BF16)
        wcs = consts.tile([128, M * U], BF16)
        wci = consts.tile([128, U], BF16)
        bg = consts.tile([128, 1], F32)
        bc = consts.tile([U, 1], F32)
        for dst, src in ((wgs, wgsd), (wgi, wgid), (wcs, wcsd), (wci, wcid),
                         (bg, bgd), (bc, bcd)):
            nc.sync.dma_start(dst[:], src[:])

        # natural-layout activations (bf16): block i at cols i*CW
        x0sb = nat.tile([128, NB * CW], BF16, tag="x0")
        x1sb = nat.tile([128, NB * CW], BF16, tag="x1")
        x0dv = x0d.rearrange("(t p) c -> t p c", p=128)
        for i in range(NB):
            nc.sync.dma_start(x0sb[:, i * CW:(i + 1) * CW], x0dv[i])

        # xs^T state parts, bf16: block (m, j) = batches {2j, 2j+1}, all n
        xsts = xst.tile([128, M * 4 * N], BF16, tag="xsts")
        # input-feature diffusion, natural gather: cols b*32 + m*D_IN + fi
        xicat = xst.tile([128, NB * 256], BF16, tag="xicat")
        nc.vector.memset(xicat[:], 0.0)
        # packed input-feature rhs: tile t, batch b=3t+k at rows 32*k
        xpk = [xst.tile([128, N], BF16, tag=f"xpk{t}", name=f"xpk{t}")
               for t in range(3)]

        def xst_s(m, j):
            return xsts[:, (m * 4 + j) * N:(m * 4 + j + 1) * N]

        def transpose_to_xst(m, i, src_ap, with_input):
            """src_ap: natural bf16 block [128, CW-ish]; writes xs^T."""
            for j in range(4):
                pt = psum.tile([128, 128], BF16, tag="ps")
                nc.tensor.transpose(
                    pt[:], src_ap[:, j * 128:(j + 1) * 128], identb[:])
                nc.vector.tensor_copy(
                    xst_s(m, j)[:, i * 128:(i + 1) * 128], pt[:])
            if with_input:
                # gather input cols into xicat (b,f)-strided -> (b,m,f)
                src3 = src_ap[:, SC:SC + IC].rearrange("p (b f) -> p b f", b=BL)
                dst3 = xicat[:, i * 256:(i + 1) * 256].rearrange(
                    "p (b r) -> p b r", r=32)[:, :, m * D_IN:(m + 1) * D_IN]
                nc.vector.tensor_copy(dst3, src3)

        def spmm(std, xsrc, chunks, dest_cb):
            """Y = S @ X (bf16). Per row-block: one column-batched S DMA,
            then K-accumulated matmuls; dest_cb(i, psum_list) consumes."""
            stdv = std.rearrange("(j p) c -> p j c", p=128)
            for i in range(NB):
                sc = spool.tile([128, NB * 128], BF16, tag="sc")
                nc.sync.dma_start(
                    sc[:].rearrange("p (j c) -> p j c", j=NB),
                    stdv[:, :, i * 128:(i + 1) * 128])
                pts = [psum.tile([128, c1 - c0], F32, tag="ps",
                                 name=f"pmm{i}_{c0}")
                       for (c0, c1) in chunks]
                for j in range(NB):
                    for ci, (c0, c1) in enumerate(chunks):
                        nc.tensor.matmul(
                            pts[ci][:], sc[:, j * 128:(j + 1) * 128],
                            xsrc[:, j * CW + c0:j * CW + c1],
                            start=(j == 0), stop=(j == NB - 1))
                dest_cb(i, pts)

        GCH = [(0, 512), (512, 528)]
        CCH = [(0, 512)]

        def dconv(xnat, x1nat, is_gate):
            """Emit one diffusion-conv's spmm + transpose stages."""
            chunks = GCH if is_gate else CCH
            wid = SC + (IC if is_gate else 0)
            for i in range(NB):
                transpose_to_xst(0, i, xnat[:, i * CW:i * CW + CW], is_gate)
            for sup, std in ((0, satd), (1, sbtd)):
                m1, m2 = 1 + 2 * sup, 2 + 2 * sup

                def x1_sink(i, pts):
                    for pt, (c0, c1) in zip(pts, chunks):
                        nc.vector.tensor_copy(
                            x1nat[:, i * CW + c0:i * CW + c1], pt[:])
                    transpose_to_xst(m1, i, x1nat[:, i * CW:i * CW + CW],
                                     is_gate)

                spmm(std, xnat, chunks, x1_sink)

                # x2' = S x1 - 0.5 x0  (W of the k=2 terms pre-doubled)
                def x2_sink(i, pts):
                    blk = x2p.tile([128, CW], BF16, tag="x2")
                    for pt, (c0, c1) in zip(pts, chunks):
                        nc.vector.scalar_tensor_tensor(
                            blk[:, c0:c1],
                            xnat[:, i * CW + c0:i * CW + c1],
                            -0.5, pt[:],
                            mybir.AluOpType.mult, mybir.AluOpType.add)
                    transpose_to_xst(m2, i, blk[:, 0:wid], is_gate)

                spmm(std, x1nat, chunks, x2_sink)

        def finalize_inputs():
            for i in range(NB):
                for t in range(3):
                    w = 96 if t < 2 else 64
                    pt = psum.tile([w, 128], BF16, tag="ps", name=f"pfin{t}")
                    nc.tensor.transpose(
                        pt[:], xicat[:, i * 256 + t * 96:i * 256 + t * 96 + w],
                        identb[:])
                    nc.vector.tensor_copy(xpk[t][:w, i * 128:(i + 1) * 128],
                                          pt[:])

        def w_stage(is_gate):
            """Projection + activation. Gate: sigmoid -> r (into x0sb),
            u (to DRAM). Cand: tanh -> c (to DRAM)."""
            ws, wi, O = (wgs, wgi, 128) if is_gate else (wcs, wci, U)
            for b in range(BL):
                t, k = b // 3, b % 3
                for c in range(4):  # n-chunks of 512
                    pt = psum.tile([O, 512], F32, tag="ps")
                    bp = (b % 2) * U
                    for m in range(M):
                        rs = xst_s(m, b // 2)[bp:bp + U, c * 512:(c + 1) * 512]
                        nc.tensor.matmul(pt[:], ws[bp:bp + U, m * O:(m + 1) * O],
                                         rs, start=(m == 0), stop=False)
                    ri = xpk[t][32 * k:32 * k + PK, c * 512:(c + 1) * 512]
                    nc.tensor.matmul(pt[:], wi[32 * k:32 * k + PK, :O], ri,
                                     start=False, stop=True)
                    h = c // 2
                    cols = slice(b * 1024 + 512 * (c % 2),
                                 b * 1024 + 512 * (c % 2) + 512)
                    if is_gate:
                        rb = small.tile([U, 512], F32, tag="rb")
                        nc.scalar.activation(rb[:], pt[:U, :],
                                             mybir.ActivationFunctionType.Sigmoid,
                                             bias=bg[:U, :])
                        ub = small.tile([U, 512], F32, tag="ub")
                        nc.scalar.activation(ub[:], pt[U:128, :],
                                             mybir.ActivationFunctionType.Sigmoid,
                                             bias=bg[U:128, :])
                        nc.sync.dma_start(ubufd[64 * h:64 * h + 64, cols], ub[:])
                        # r^T into x0 state cols (candidate input, in place)
                        rpt = psum.tile([128, 256], F32, tag="ps")
                        for j in range(4):
                            nc.tensor.transpose(
                                rpt[:, j * U:(j + 1) * U],
                                rb[:, j * 128:(j + 1) * 128], ident[:U, :U])
                        xv = x0sb[:].rearrange("p (i c) -> p i c", c=CW)[
                            :, 4 * c:4 * c + 4, b * U:(b + 1) * U]
                        nc.vector.tensor_mul(
                            xv, xv,
                            rpt[:].rearrange("p (i o) -> p i o", o=U))
                    else:
                        cb = small.tile([U, 512], F32, tag="cb")
                        nc.scalar.activation(cb[:], pt[:, :],
                                             mybir.ActivationFunctionType.Tanh,
                                             bias=bc[:])
                        nc.sync.dma_start(cbufd[64 * h:64 * h + 64, cols], cb[:])

        def final():
            """new_state = c + u*(state - c), natural layout, batched per
            (b, half). u/c come back [64, 1024]; state fp32 re-read."""
            stfv = stfd.rearrange("(i p) c -> p i c", p=128)
            outv = outd.rearrange("(i p) c -> p i c", p=128)
            for b in range(BL):
                for h in range(2):
                    i0 = h * 8
                    ut = small.tile([U, 1024], F32, tag="ut")
                    nc.sync.dma_start(
                        ut[:], ubufd[64 * h:64 * h + 64,
                                     b * 1024:(b + 1) * 1024])
                    ct = small.tile([U, 1024], F32, tag="ct")
                    nc.sync.dma_start(
                        ct[:], cbufd[64 * h:64 * h + 64,
                                     b * 1024:(b + 1) * 1024])
                    stt = small.tile([128, 512], F32, tag="stt")
                    nc.sync.dma_start(
                        stt[:].rearrange("p (i c) -> p i c", c=U),
                        stfv[:, i0:i0 + 8, b * U:(b + 1) * U])
                    cpt = psum.tile([128, 512], F32, tag="ps")
                    upt = psum.tile([128, 512], F32, tag="ps")
                    for j in range(8):
                        nc.tensor.transpose(cpt[:, j * U:(j + 1) * U],
                                            ct[:, j * 128:(j + 1) * 128],
                                            ident[:U, :U])
                        nc.tensor.transpose(upt[:, j * U:(j + 1) * U],
                                            ut[:, j * 128:(j + 1) * 128],
                                            ident[:U, :U])
                    # stt = (stt - c) * u + c, all [128, 512], in place
                    nc.vector.tensor_sub(stt[:], stt[:], cpt[:])
                    nc.vector.tensor_mul(stt[:], stt[:], upt[:])
                    nc.vector.tensor_add(stt[:], stt[:], cpt[:])
                    nc.sync.dma_start(
                        outv[:, i0:i0 + 8, b * U:(b + 1) * U],
                        stt[:].rearrange("p (i c) -> p i c", c=U))

        # ---- gate dconv ----
        dconv(x0sb, x1sb, True)
        finalize_inputs()
        w_stage(True)
        # ---- candidate dconv (x0sb is now candX in its state cols) ----
        dconv(x0sb, x1sb, False)
        w_stage(False)
        final()


_NC_CACHE = {}


def _get_nc():
    if "nc" not in _NC_CACHE:
        _NC_CACHE["nc"] = _build_nc()
    return _NC_CACHE["nc"]


def _host_prep(inputs, state, edges1, vals1, edges2, vals2, W_gate, b_gate,
               W_cand, b_cand):
    import ml_dtypes
    BF = ml_dtypes.bfloat16
    inputs = np.asarray(inputs, np.float32)
    state = np.asarray(state, np.float32)

    def densify_T(edges, vals):
        ST = np.zeros((N, N), np.float32)
        np.add.at(ST, (np.asarray(edges[1]).astype(np.int64),
                       np.asarray(edges[0]).astype(np.int64)),
                  np.asarray(vals, np.float32))
        return ST.astype(BF)

    SaT = densify_T(edges1, vals1)
    SbT = densify_T(edges2, vals2)

    def reorder(Wmat):
        Wmat = np.asarray(Wmat, np.float32)
        O = Wmat.shape[1]
        Wm = Wmat.reshape(F, M, O).copy()
        Wm[:, 2, :] *= 2.0
        Wm[:, 4, :] *= 2.0
        # state rows duplicated at partition bases 0 and 64
        Ws = np.ascontiguousarray(Wm[D_IN:].reshape(U, M * O))
        Ws2 = np.concatenate([Ws, Ws], 0)                       # [128, M*O]
        # input rows (m, fi) packed [10, O], replicated at bases 0/32/64
        Wi = np.ascontiguousarray(Wm[:D_IN].transpose(1, 0, 2).reshape(PK, O))
        Wi2 = np.zeros((128, O), np.float32)
        for base in (0, 32, 64):
            Wi2[base:base + PK] = Wi
        return (Ws2.astype(BF), Wi2.astype(BF))

    wgs, wgi = reorder(W_gate)
    wcs, wci = reorder(W_cand)
    bg = np.asarray(b_gate, np.float32).reshape(128, 1)
    bc = np.asarray(b_cand, np.float32).reshape(U, 1)

    in_maps = []
    for c in range(NCORES):
        bsl = slice(c * BL, (c + 1) * BL)
        st_c = state[bsl].reshape(BL, N, U)
        in_c = inputs[bsl].reshape(BL, N, D_IN)
        statef = np.ascontiguousarray(st_c.transpose(1, 0, 2).reshape(N, SC))
        x0 = np.empty((N, CW), np.float32)
        x0[:, :SC] = statef
        x0[:, SC:] = in_c.transpose(1, 0, 2).reshape(N, IC)
        in_maps.append(dict(x0=x0.astype(BF), statef=statef, sat=SaT,
                            sbt=SbT, wgs=wgs, wgi=wgi, wcs=wcs, wci=wci,
                            bg=bg, bc=bc))
    return in_maps


def kernel(**inputs):
    nc = _get_nc()
    in_maps = _host_prep(**inputs)
    res = run_bass_kernel_spmd(nc, in_maps, list(range(NCORES)))
    outs = []
    for c in range(NCORES):
        o = np.asarray(res.results[c]["out"])          # [N, (b, u)]
        outs.append(o.reshape(N, BL, U).transpose(1, 0, 2).reshape(BL, N * U))
    return np.concatenate(outs, 0).astype(np.float32)



# revision 6
# speedup vs baseline: 1.7860x; 1.7860x over previous
"""DCGRU cell on 8 Trainium2 NeuronCores.

Strategy (data-parallel over batch B=64 -> 8 per core):
  - Sparse supports densified on host into S^T [2048, 2048], pre-scaled
    x512 and quantized to fp8e4 (e4m3); both supports stay SBUF-resident
    (loaded once).  All diffusion spmms run as fp8 DoubleRow matmuls
    (K=256 per instruction at 0.5 cyc/row, fp32 PSUM accumulate).
  - Activations live node-major [n, (b,f)]: x0 in bf16 (+ a host-made
    fp8 x16 copy); x1 is stored ONLY as fp8(16*x1) (scalar-engine
    PSUM evacuation with scale 1/512); the Chebyshev term is kept as
    8192*x2' = S8@x1q - 4096*x0 in bf16 (one scalar_tensor_tensor).
    All scales are exact powers of two folded into the host-prepped W.
  - Diffusion outputs are PE-transposed to feature-major xs^T tiles
    (bf16 for m0/m2/m4, fp8 for m1/m3) for the projection; input
    features go through a gathered xicat -> packed xpk path (K=10).
  - Sink stages are software-pipelined one block behind the matmuls so
    the PE never stalls on Scalar/Vector PSUM evacuations.
  - Gate: r^T is multiplied into x0 in place (candidate input built in
    situ), u^T goes to an SBUF-resident natural tile.  The candidate
    w_stage fuses the GRU combine: per (b, n-half) it transposes c,
    streams the bf16 state, and writes the output directly -- no DRAM
    round-trips for u/c.
"""

import numpy as np

import concourse.bass as bass
from concourse import bacc
import concourse.mybir as mybir
import concourse.tile as tile
from concourse.bass_utils import run_bass_kernel_spmd
from concourse.masks import make_identity

N = 2048            # nodes
B = 64              # global batch
BL = 8              # batch per core
NCORES = 8
D_IN = 2
U = 64              # hidden units
M = 5               # 1 + 2 supports * K
F = D_IN + U        # 66
NB = N // 128       # 16 node blocks
SC = BL * U         # 512 state cols in natural layout
IC = BL * D_IN      # 16 input cols
CW = SC + IC        # 528 total natural cols
PK = M * D_IN       # 10 packed input rows per batch

F32 = mybir.dt.float32
BF16 = mybir.dt.bfloat16
FP8 = mybir.dt.float8e4

S_SCALE = 512.0     # S^T fp8 pre-scale (host)
X_SCALE = 16.0      # x fp8 pre-scale
PROD = S_SCALE * X_SCALE   # psum scale of S8 @ xq


def _build_nc():
    nc = bacc.Bacc(None, target_bir_lowering=False)

    x0d = nc.declare_dram_parameter("x0", [N, CW], BF16, isOutput=False)
    x0qd = nc.declare_dram_parameter("x0q", [N, CW], FP8, isOutput=False)
    sad = nc.declare_dram_parameter("sat8", [N, N], FP8, isOutput=False)
    sbd = nc.declare_dram_parameter("sbt8", [N, N], FP8, isOutput=False)
    stfd = nc.declare_dram_parameter("statef", [N, SC], BF16, isOutput=False)
    wgsd = nc.declare_dram_parameter("wgs", [128, M * 128], BF16, isOutput=False)
    wgid = nc.declare_dram_parameter("wgi", [128, 128], BF16, isOutput=False)
    wcsd = nc.declare_dram_parameter("wcs", [128, M * U], BF16, isOutput=False)
    wcid = nc.declare_dram_parameter("wci", [128, U], BF16, isOutput=False)
    bgd = nc.declare_dram_parameter("bg", [128, 1], F32, isOutput=False)
    bcd = nc.declare_dram_parameter("bc", [U, 1], F32, isOutput=False)
    outd = nc.declare_dram_parameter("out", [N, SC], F32, isOutput=True)
    ubufd = nc.dram_tensor("ubuf", [128, NB * SC], BF16)

    with tile.TileContext(nc) as tc:
        _emit(nc, tc, x0d, x0qd, sad, sbd, stfd, wgsd, wgid, wcsd, wcid,
              bgd, bcd, outd, ubufd)
    nc.compile()
    return nc


def _emit(nc, tc, x0d, x0qd, sad, sbd, stfd, wgsd, wgid, wcsd, wcid, bgd,
          bcd, outd, ubufd):
    from contextlib import ExitStack
    ctx = ExitStack()
    with ctx:
        consts = ctx.enter_context(tc.tile_pool(name="consts", bufs=1))
        nat = ctx.enter_context(tc.tile_pool(name="nat", bufs=1))
        x2p = ctx.enter_context(tc.tile_pool(name="x2p", bufs=2))
        small = ctx.enter_context(tc.tile_pool(name="small", bufs=2))
        cbp = ctx.enter_context(tc.tile_pool(name="cbp", bufs=4))
        stp = ctx.enter_context(tc.tile_pool(name="stp", bufs=2))
        utp = ctx.enter_context(tc.tile_pool(name="utp", bufs=2))
        ubp = ctx.enter_context(tc.tile_pool(name="ubp", bufs=2))
        tfp = ctx.enter_context(tc.tile_pool(name="tfp", bufs=2))
        psum = ctx.enter_context(tc.tile_pool(name="psum", bufs=8, space="PSUM"))

        identb = consts.tile([128, 128], BF16)
        make_identity(nc, identb[:])
        ident8 = consts.tile([128, 128], FP8)
        nc.vector.tensor_copy(ident8[:], identb[:])

        wgs = consts.tile([128, M * 128], BF16)
        wgi = consts.tile([128, 128], BF16)
        wcs = consts.tile([128, M * U], BF16)
        wci = consts.tile([128, U], BF16)
        bg = consts.tile([128, 1], F32)
        bc = consts.tile([U, 1], F32)

        # natural-layout activations: block i at cols i*CW
        x0sb = nat.tile([128, NB * CW], BF16, tag="x0")
        x0q = nat.tile([128, NB * CW], FP8, tag="x0q")
        x1q = nat.tile([128, NB * CW], FP8, tag="x1q")
        sa8 = nat.tile([128, NB * N], FP8, tag="sa8")
        sb8 = nat.tile([128, NB * N], FP8, tag="sb8")
        # xs^T: bf16 for m in {0, 2, 4} (idx 0,1,2), fp8 for {1, 3} (idx 0,1)
        xsts = nat.tile([128, 3 * 4 * N], BF16, tag="xsts")
        xsts8 = nat.tile([128, 2 * 4 * N], FP8, tag="xsts8")
        # input-feature gather: block i cols i*256, within: b*32 + m*2 + fi
        xicat = nat.tile([128, NB * 256], BF16, tag="xicat")
        # packed input rhs: tile t holds batches 3t+k at rows 32k
        xpk = [nat.tile([128, N], BF16, tag=f"xpk{t}", name=f"xpk{t}")
               for t in range(3)]

        x0dv = x0d.rearrange("(t p) c -> t p c", p=128)
        x0qdv = x0qd.rearrange("(j p) c -> p j c", p=128)
        for i in range(NB):
            nc.sync.dma_start(x0sb[:, i * CW:(i + 1) * CW], x0dv[i])
        nc.sync.dma_start(
            x0q[:].rearrange("p (j c) -> p j c", j=NB), x0qdv)
        nc.sync.dma_start(
            sa8[:].rearrange("p (j c) -> p j c", j=NB),
            sad.rearrange("(j p) c -> p j c", p=128))
        nc.sync.dma_start(
            sb8[:].rearrange("p (j c) -> p j c", j=NB),
            sbd.rearrange("(j p) c -> p j c", p=128))
        for dst, src in ((wgs, wgsd), (wgi, wgid), (wcs, wcsd), (wci, wcid),
                         (bg, bgd), (bc, bcd)):
            nc.sync.dma_start(dst[:], src[:])
        nc.vector.memset(xicat[:], 0.0)

        stfv = stfd.rearrange("(i p) c -> p i c", p=128)
        outv = outd.rearrange("(i p) c -> p i c", p=128)
        x1q3 = x1q[:].rearrange("p (j c) -> p j c", j=NB)

        def xst_s(mi, j):
            return xsts[:, (mi * 4 + j) * N:(mi * 4 + j + 1) * N]

        def xst8_s(mi, j):
            return xsts8[:, (mi * 4 + j) * N:(mi * 4 + j + 1) * N]

        def xst_transposes(sel, i, src_ap):
            """4 PE transposes of natural block i state cols into xs^T.
            sel = (is_fp8, mi)."""
            is8, mi = sel
            idt = ident8 if is8 else identb
            dst = xst8_s if is8 else xst_s
            for j in range(4):
                if is8:
                    # fp8 transpose requires psum element step of 2
                    pt = psum.tile([128, 256], FP8, tag="ps")
                    pv = pt[:].rearrange("p (c two) -> p c two", two=2)[:, :, 0]
                else:
                    pt = psum.tile([128, 128], BF16, tag="ps")
                    pv = pt[:]
                nc.tensor.transpose(
                    pv, src_ap[:, j * 128:(j + 1) * 128], idt[:])
                nc.vector.tensor_copy(
                    dst(mi, j)[:, i * 128:(i + 1) * 128], pv)

        def xicat_gather(m, i, src_ap):
            """Copy input cols (b,f) of natural block -> xicat (b, m, f)."""
            src3 = src_ap[:, SC:SC + IC].rearrange("p (b f) -> p b f", b=BL)
            dst3 = xicat[:, i * 256:(i + 1) * 256].rearrange(
                "p (b r) -> p b r", r=32)[:, :, m * D_IN:(m + 1) * D_IN]
            nc.vector.tensor_copy(dst3, src3)

        def spmm(s8, xq, chunks, sink):
            """Y = S8 @ Xq via fp8 DoubleRow (K=256/instr), fp32 PSUM.
            sink(i, pts) emits evacuation; its PE work is pipelined one
            block behind by the caller through the returned closures."""
            s3 = s8[:].rearrange("p (j c) -> p j c", j=NB)
            xq3 = xq[:].rearrange("p (j c) -> p j c", j=NB)
            pending = None
            for i in range(NB):
                pts = [psum.tile([128, c1 - c0], F32, tag="ps",
                                 name=f"pmm{i}_{c0}")
                       for (c0, c1) in chunks]
                for jj in range(NB // 2):
                    for ci, (c0, c1) in enumerate(chunks):
                        nc.tensor.matmul(
                            pts[ci][:],
                            s3[:, 2 * jj:2 * jj + 2, i * 128:(i + 1) * 128],
                            xq3[:, 2 * jj:2 * jj + 2, c0:c1],
                            start=(jj == 0), stop=(jj == NB // 2 - 1),
                            perf_mode=mybir.MatmulPerfMode.DoubleRow)
                if pending is not None:
                    pending()
                pending = sink(i, pts)
            pending()

        GCH = [(0, 512), (512, 528)]
        CCH = [(0, 512)]

        def dconv(gate):
            chunks = GCH if gate else CCH
            for i in range(NB):
                xst_transposes((False, 0), i, x0sb[:, i * CW:i * CW + SC])
                if gate:
                    xicat_gather(0, i, x0sb[:, i * CW:(i + 1) * CW])
            for s, s8 in ((0, sa8), (1, sb8)):

                def x1_sink(i, pts, s=s):
                    xb = x1q3[:, i]
                    for pt, (c0, c1) in zip(pts, chunks):
                        nc.scalar.mul(xb[:, c0:c1], pt[:], 1.0 / S_SCALE)

                    def deferred():
                        xst_transposes((True, s), i, xb)
                        if gate:
                            xicat_gather(1 + 2 * s, i,
                                         x1q[:, i * CW:(i + 1) * CW])
                    return deferred

                # NOTE: x1q blocks are CW-strided only via x1q3 view; the
                # gather wants the block AP, so pass the x1q slice there.
                spmm(s8, x0q, chunks, x1_sink)

                def x2_sink(i, pts, s=s):
                    blk = x2p.tile([128, CW], BF16, tag="x2")
                    for pt, (c0, c1) in zip(pts, chunks):
                        nc.vector.scalar_tensor_tensor(
                            blk[:, c0:c1],
                            x0sb[:, i * CW + c0:i * CW + c1],
                            -(PROD / 2.0), pt[:],
                            mybir.AluOpType.mult, mybir.AluOpType.add)

                    def deferred():
                        xst_transposes((False, 1 + s), i, blk)
                        if gate:
                            xicat_gather(2 + 2 * s, i, blk)
                    return deferred

                spmm(s8, x1q, chunks, x2_sink)

        def finalize_inputs():
            for i in range(NB):
                for t in range(3):
                    w = 96 if t < 2 else 64
                    pt = psum.tile([w, 128], BF16, tag="ps", name=f"pf{t}")
                    nc.tensor.transpose(
                        pt[:], xicat[:, i * 256 + t * 96:i * 256 + t * 96 + w],
                        identb[:])
                    nc.vector.tensor_copy(xpk[t][:w, i * 128:(i + 1) * 128],
                                          pt[:])

        XSRC = [(False, 0), (True, 0), (False, 1), (True, 1), (False, 2)]
        ubv = ubufd.rearrange("p (i b u) -> p i b u", b=BL, u=U)

        def proj_mms(b, c, gate):
            ws, wi, O = (wgs, wgi, 128) if gate else (wcs, wci, U)
            t, k = b // 3, b % 3
            pt = psum.tile([O, 512], F32, tag="ps", name="po")
            bp = (b % 2) * U
            for m in range(M):
                is8, mi = XSRC[m]
                src = (xst8_s if is8 else xst_s)(mi, b // 2)
                rs = src[bp:bp + U, c * 512:(c + 1) * 512]
                nc.tensor.matmul(pt[:], ws[bp:bp + U, m * O:(m + 1) * O],
                                 rs, start=(m == 0), stop=False)
            ri = xpk[t][32 * k:32 * k + PK, c * 512:(c + 1) * 512]
            nc.tensor.matmul(pt[:], wi[32 * k:32 * k + PK, :O], ri,
                             start=False, stop=True)
            return pt

        def w_stage_gate():
            """sigmoid -> r (x0sb *= r^T in place), u -> u_nat (SBUF)."""
            pending = None
            for b in range(BL):
                for c in range(4):
                    pt = proj_mms(b, c, True)
                    rb = small.tile([U, 512], BF16, tag="rb")
                    nc.scalar.activation(rb[:], pt[:U, :],
                                         mybir.ActivationFunctionType.Sigmoid,
                                         bias=bg[:U, :])
                    ub = small.tile([U, 512], BF16, tag="ub")
                    nc.scalar.activation(ub[:], pt[U:128, :],
                                         mybir.ActivationFunctionType.Sigmoid,
                                         bias=bg[U:128, :])

                    def deferred(b=b, c=c, rb=rb, ub=ub):
                        rpt = psum.tile([128, 256], BF16, tag="ps")
                        upt = psum.tile([128, 256], BF16, tag="ps")
                        for j in range(4):
                            nc.tensor.transpose(
                                rpt[:, j * U:(j + 1) * U],
                                rb[:, j * 128:(j + 1) * 128], identb[:U, :U])
                            nc.tensor.transpose(
                                upt[:, j * U:(j + 1) * U],
                                ub[:, j * 128:(j + 1) * 128], identb[:U, :U])
                        xv = x0sb[:].rearrange("p (i c) -> p i c", c=CW)[
                            :, 4 * c:4 * c + 4, b * U:(b + 1) * U]
                        nc.vector.tensor_mul(
                            xv, xv,
                            rpt[:].rearrange("p (i o) -> p i o", o=U))
                        ubb = ubp.tile([128, 256], BF16, tag="ubb")
                        nc.vector.tensor_copy(ubb[:], upt[:])
                        nc.sync.dma_start(
                            ubv[:, 4 * c:4 * c + 4, b, :],
                            ubb[:].rearrange("p (i o) -> p i o", o=U))
                    if pending is not None:
                        pending()
                    pending = deferred
            pending()

        def w_stage_cand():
            """tanh -> c; fused GRU combine per (b, n-half)."""
            pending = None
            for b in range(BL):
                for h in range(2):
                    stt = stp.tile([128, 512], BF16, tag="stt")
                    nc.sync.dma_start(
                        stt[:].rearrange("p (i u) -> p i u", u=U),
                        stfv[:, 8 * h:8 * h + 8, b * U:(b + 1) * U])
                    ut = utp.tile([128, 512], BF16, tag="ut")
                    nc.sync.dma_start(
                        ut[:].rearrange("p (i u) -> p i u", u=U),
                        ubv[:, 8 * h:8 * h + 8, b, :])
                    cbs = []
                    for cc in range(2):
                        pt = proj_mms(b, 2 * h + cc, False)
                        cb = cbp.tile([U, 512], BF16, tag="cb")
                        nc.scalar.activation(
                            cb[:], pt[:, :],
                            mybir.ActivationFunctionType.Tanh, bias=bc[:])
                        cbs.append(cb)

                    def deferred(b=b, h=h, cbs=cbs, stt=stt, ut=ut):
                        cpt = psum.tile([128, 512], BF16, tag="ps")
                        for cc in range(2):
                            for j in range(4):
                                nc.tensor.transpose(
                                    cpt[:, cc * 256 + j * U:
                                        cc * 256 + (j + 1) * U],
                                    cbs[cc][:, j * 128:(j + 1) * 128],
                                    identb[:U, :U])
                        tf = tfp.tile([128, 512], F32, tag="tf")
                        # tf = (state - c) * u + c
                        nc.vector.tensor_sub(tf[:], stt[:], cpt[:])
                        nc.vector.tensor_mul(tf[:], tf[:], ut[:])
                        nc.vector.tensor_add(tf[:], tf[:], cpt[:])
                        nc.sync.dma_start(
                            outv[:, 8 * h:8 * h + 8, b * U:(b + 1) * U],
                            tf[:].rearrange("p (i u) -> p i u", u=U))
                    if pending is not None:
                        pending()
                    pending = deferred
            pending()

        # ---- gate ----
        dconv(True)
        finalize_inputs()
        w_stage_gate()
        # ---- candidate (x0sb now holds candX in its state cols) ----
        nc.scalar.mul(x0q[:], x0sb[:], X_SCALE)
        dconv(False)
        w_stage_cand()


_NC_CACHE = {}


def _get_nc():
    if "nc" not in _NC_CACHE:
        _NC_CACHE["nc"] = _build_nc()
    return _NC_CACHE["nc"]


def _host_prep(inputs, state, edges1, vals1, edges2, vals2, W_gate, b_gate,
               W_cand, b_cand):
    import ml_dtypes
    BF = ml_dtypes.bfloat16
    E4 = ml_dtypes.float8_e4m3
    inputs = np.asarray(inputs, np.float32)
    state = np.asarray(state, np.float32)

    def densify_T8(edges, vals):
        ST = np.zeros((N, N), np.float32)
        np.add.at(ST, (np.asarray(edges[1]).astype(np.int64),
                       np.asarray(edges[0]).astype(np.int64)),
                  np.asarray(vals, np.float32))
        return (ST * S_SCALE).astype(E4)

    SaT8 = densify_T8(edges1, vals1)
    SbT8 = densify_T8(edges2, vals2)

    # per-m scales folded into W (exact powers of two):
    #   m0: 1; m1/m3 (x1 stored as 16*x1): 1/16;
    #   m2/m4 (stored as 8192*x2', Chebyshev needs 2*x2'): 2/8192.
    MSC = [1.0, 1.0 / X_SCALE, 2.0 / PROD, 1.0 / X_SCALE, 2.0 / PROD]

    def reorder(Wmat):
        Wmat = np.asarray(Wmat, np.float32)
        O = Wmat.shape[1]
        Wm = Wmat.reshape(F, M, O).copy()
        for m in range(M):
            Wm[:, m, :] *= MSC[m]
        # state rows duplicated at partition bases 0 and 64
        Ws = np.ascontiguousarray(Wm[D_IN:].reshape(U, M * O))
        Ws2 = np.concatenate([Ws, Ws], 0)                       # [128, M*O]
        # input rows (m, fi) packed [10, O], replicated at bases 0/16/32
        Wi = np.ascontiguousarray(Wm[:D_IN].transpose(1, 0, 2).reshape(PK, O))
        Wi2 = np.zeros((128, O), np.float32)
        for base in (0, 32, 64):
            Wi2[base:base + PK] = Wi
        return (Ws2.astype(BF), Wi2.astype(BF))

    wgs, wgi = reorder(W_gate)
    wcs, wci = reorder(W_cand)
    bg = np.asarray(b_gate, np.float32).reshape(128, 1)
    bc = np.asarray(b_cand, np.float32).reshape(U, 1)

    in_maps = []
    for c in range(NCORES):
        bsl = slice(c * BL, (c + 1) * BL)
        st_c = state[bsl].reshape(BL, N, U)
        in_c = inputs[bsl].reshape(BL, N, D_IN)
        statef = np.ascontiguousarray(st_c.transpose(1, 0, 2).reshape(N, SC))
        x0 = np.empty((N, CW), np.float32)
        x0[:, :SC] = statef
        x0[:, SC:] = in_c.transpose(1, 0, 2).reshape(N, IC)
        in_maps.append(dict(x0=x0.astype(BF),
                            x0q=(x0 * X_SCALE).astype(E4),
                            statef=statef.astype(BF),
                            sat8=SaT8, sbt8=SbT8, wgs=wgs, wgi=wgi,
                            wcs=wcs, wci=wci, bg=bg, bc=bc))
    return in_maps


def kernel(**inputs):
    nc = _get_nc()
    in_maps = _host_prep(**inputs)
    res = run_bass_kernel_spmd(nc, in_maps, list(range(NCORES)))
    outs = []
    for c in range(NCORES):
        o = np.asarray(res.results[c]["out"])          # [N, (b, u)]
        outs.append(o.reshape(N, BL, U).transpose(1, 0, 2).reshape(BL, N * U))
    return np.concatenate(outs, 0).astype(np.float32)


# revision 9
# speedup vs baseline: 2.1068x; 1.1796x over previous
"""DCGRU cell on 8 Trainium2 NeuronCores.

Strategy (data-parallel over batch B=64 -> 8 per core):
  - Sparse supports densified on host into S^T [2048, 2048], pre-scaled
    x512 and quantized to fp8e4 (e4m3); both supports stay SBUF-resident
    (loaded once, column-chunked DMAs so the first spmm starts early).
    All diffusion spmms run as fp8 DoubleRow matmuls (K=256 per
    instruction, fp32 PSUM accumulate).
  - Input features never touch the device diffusion: their projection
    contribution (identical for gate and candidate) is computed on the
    host in f32 and DMA-preloaded into each projection PSUM tile
    (matmuls then accumulate with start=False).
  - Activations live node-major [n, (b,u)], 512 cols: x0 in bf16 (+ a
    host-made fp8 x16 copy); x1 is stored ONLY as fp8(16*x1); the
    Chebyshev term is kept as 8192*x2' = S8@x1q - 4096*x0 in bf16.
  - Projection W is host-scaled by 256 (activation applies scale=1/256),
    which puts the fp8 copy of the m1/m3 rows (16*W) in prime e4m3
    range: m1+m3 contract as ONE fp8 DoubleRow matmul against the
    fp8 xs^T tiles (values x16, scales cancel by construction).
    Per (b, n-chunk): preload-DMA + 3 bf16 K=64 matmuls + 1 fp8
    DoubleRow K=128.
  - Sink stages are software-pipelined one block behind the matmuls so
    the PE never stalls on Scalar/Vector PSUM evacuations.
  - Gate: r^T is multiplied into x0 in place, u^T round-trips DRAM in
    bf16 (prefetched back).  The candidate w_stage fuses the GRU
    combine: per (b, n-half) it transposes c, streams the bf16 state
    (same DRAM tensor as x0), and writes the output directly.
"""

import numpy as np

import concourse.bass as bass
from concourse import bacc
import concourse.mybir as mybir
import concourse.tile as tile
from concourse.bass_utils import run_bass_kernel_spmd
from concourse.masks import make_identity

N = 2048            # nodes
B = 64              # global batch
BL = 8              # batch per core
NCORES = 8
D_IN = 2
U = 64              # hidden units
M = 5               # 1 + 2 supports * K
F = D_IN + U        # 66
NB = N // 128       # 16 node blocks
SC = BL * U         # 512 state cols in natural layout

F32 = mybir.dt.float32
BF16 = mybir.dt.bfloat16
FP8 = mybir.dt.float8e4

S_SCALE = 512.0     # S^T fp8 pre-scale (host)
X_SCALE = 16.0      # x fp8 pre-scale
PROD = S_SCALE * X_SCALE   # psum scale of S8 @ xq
W_SCALE = 256.0     # projection W pre-scale (activation descales)


def _build_nc():
    nc = bacc.Bacc(None, target_bir_lowering=False)

    x0d = nc.declare_dram_parameter("x0", [N, SC], BF16, isOutput=False)
    x0qd = nc.declare_dram_parameter("x0q", [N, SC], FP8, isOutput=False)
    sad = nc.declare_dram_parameter("sat8", [N, N], FP8, isOutput=False)
    sbd = nc.declare_dram_parameter("sbt8", [N, N], FP8, isOutput=False)
    wgsd = nc.declare_dram_parameter("wgs", [128, 3 * 128], BF16, isOutput=False)
    wg8d = nc.declare_dram_parameter("wg8", [128, 2 * 128], FP8, isOutput=False)
    wcsd = nc.declare_dram_parameter("wcs", [128, 3 * U], BF16, isOutput=False)
    wc8d = nc.declare_dram_parameter("wc8", [128, 2 * U], FP8, isOutput=False)
    bgd = nc.declare_dram_parameter("bg", [128, 1], F32, isOutput=False)
    bcd = nc.declare_dram_parameter("bc", [U, 1], F32, isOutput=False)
    pgid = nc.declare_dram_parameter("pgi", [BL * 128, N], F32, isOutput=False)
    pcid = nc.declare_dram_parameter("pci", [BL * U, N], F32, isOutput=False)
    outd = nc.declare_dram_parameter("out", [N, SC], F32, isOutput=True)
    ubufd = nc.dram_tensor("ubuf", [128, NB * SC], BF16)

    with tile.TileContext(nc) as tc:
        _emit(nc, tc, x0d, x0qd, sad, sbd, wgsd, wg8d, wcsd, wc8d,
              bgd, bcd, pgid, pcid, outd, ubufd)
    nc.compile()
    return nc


def _emit(nc, tc, x0d, x0qd, sad, sbd, wgsd, wg8d, wcsd, wc8d, bgd, bcd,
          pgid, pcid, outd, ubufd):
    from contextlib import ExitStack
    ctx = ExitStack()
    with ctx:
        consts = ctx.enter_context(tc.tile_pool(name="consts", bufs=1))
        nat = ctx.enter_context(tc.tile_pool(name="nat", bufs=1))
        x2p = ctx.enter_context(tc.tile_pool(name="x2p", bufs=2))
        small = ctx.enter_context(tc.tile_pool(name="small", bufs=2))
        cbp = ctx.enter_context(tc.tile_pool(name="cbp", bufs=4))
        stp = ctx.enter_context(tc.tile_pool(name="stp", bufs=2))
        utp = ctx.enter_context(tc.tile_pool(name="utp", bufs=2))
        ubp = ctx.enter_context(tc.tile_pool(name="ubp", bufs=2))
        tfp = ctx.enter_context(tc.tile_pool(name="tfp", bufs=2))
        pinp = ctx.enter_context(tc.tile_pool(name="pinp", bufs=4))
        psum = ctx.enter_context(tc.tile_pool(name="psum", bufs=8, space="PSUM"))

        identb = consts.tile([128, 128], BF16)
        make_identity(nc, identb[:])
        ident8 = consts.tile([128, 128], FP8)
        nc.vector.tensor_copy(ident8[:], identb[:])

        wgs = consts.tile([128, 3 * 128], BF16)
        wg8 = consts.tile([128, 2 * 128], FP8)
        wcs = consts.tile([128, 3 * U], BF16)
        wc8 = consts.tile([128, 2 * U], FP8)
        bg = consts.tile([128, 1], F32)
        bc = consts.tile([U, 1], F32)

        # natural-layout activations: block i at cols i*SC
        x0sb = nat.tile([128, NB * SC], BF16, tag="x0")
        x0q = nat.tile([128, NB * SC], FP8, tag="x0q")
        x1q = nat.tile([128, NB * SC], FP8, tag="x1q")
        sa8 = nat.tile([128, NB * N], FP8, tag="sa8")
        sb8 = nat.tile([128, NB * N], FP8, tag="sb8")
        # xs^T: bf16 for m in {0, 2, 4} (idx 0,1,2), fp8 for {1, 3} (idx 0,1)
        xsts = nat.tile([128, 3 * 4 * N], BF16, tag="xsts")
        xsts8 = nat.tile([128, 2 * 4 * N], FP8, tag="xsts8")

        # startup DMAs: x0 first (feeds m0 transposes), S in column chunks
        x0dv = x0d.rearrange("(j p) c -> p j c", p=128)
        x0sb3 = x0sb[:].rearrange("p (j c) -> p j c", j=NB)
        for g in range(4):
            nc.sync.dma_start(x0sb3[:, 4 * g:4 * g + 4, :],
                              x0dv[:, 4 * g:4 * g + 4, :])
        nc.sync.dma_start(
            x0q[:].rearrange("p (j c) -> p j c", j=NB),
            x0qd.rearrange("(j p) c -> p j c", p=128))
        sa3 = sa8[:].rearrange("p (j c) -> p j c", j=NB)
        sadv = sad.rearrange("(j p) c -> p j c", p=128)
        sb3 = sb8[:].rearrange("p (j c) -> p j c", j=NB)
        sbdv = sbd.rearrange("(j p) c -> p j c", p=128)
        for g in range(4):
            nc.sync.dma_start(sa3[:, :, 512 * g:512 * g + 512],
                              sadv[:, :, 512 * g:512 * g + 512])
        for g in range(4):
            nc.sync.dma_start(sb3[:, :, 512 * g:512 * g + 512],
                              sbdv[:, :, 512 * g:512 * g + 512])
        for dst, src in ((wgs, wgsd), (wg8, wg8d), (wcs, wcsd), (wc8, wc8d),
                         (bg, bgd), (bc, bcd)):
            nc.sync.dma_start(dst[:], src[:])

        stfv = x0d.rearrange("(i p) c -> p i c", p=128)
        outv = outd.rearrange("(i p) c -> p i c", p=128)
        x1q3 = x1q[:].rearrange("p (j c) -> p j c", j=NB)
        xst83 = xsts8[:].rearrange("p (mi j n) -> p mi j n", mi=2, n=N)
        wg83 = wg8[:].rearrange("p (two o) -> p two o", two=2)
        wc83 = wc8[:].rearrange("p (two o) -> p two o", two=2)

        def xst_s(mi, j):
            return xsts[:, (mi * 4 + j) * N:(mi * 4 + j + 1) * N]

        def xst8_s(mi, j):
            return xsts8[:, (mi * 4 + j) * N:(mi * 4 + j + 1) * N]

        def xst_transposes(sel, i, src_ap):
            """4 PE transposes of natural block i into xs^T."""
            is8, mi = sel
            idt = ident8 if is8 else identb
            dst = xst8_s if is8 else xst_s
            for j in range(4):
                if is8:
                    # fp8 transpose requires psum element step of 2
                    pt = psum.tile([128, 256], FP8, tag="ps")
                    pv = pt[:].rearrange("p (c two) -> p c two", two=2)[:, :, 0]
                else:
                    pt = psum.tile([128, 128], BF16, tag="ps")
                    pv = pt[:]
                nc.tensor.transpose(
                    pv, src_ap[:, j * 128:(j + 1) * 128], idt[:])
                nc.vector.tensor_copy(
                    dst(mi, j)[:, i * 128:(i + 1) * 128], pv)

        def spmm(s8, xq, sink):
            """Y = S8 @ Xq via fp8 DoubleRow (K=256/instr), fp32 PSUM.
            sink(i, pt) -> deferred PE work, pipelined one block behind."""
            s3 = s8[:].rearrange("p (j c) -> p j c", j=NB)
            xq3 = xq[:].rearrange("p (j c) -> p j c", j=NB)
            pending = None
            for i in range(NB):
                pt = psum.tile([128, 512], F32, tag="ps", name=f"pmm{i}")
                for jj in range(NB // 2):
                    nc.tensor.matmul(
                        pt[:],
                        s3[:, 2 * jj:2 * jj + 2, i * 128:(i + 1) * 128],
                        xq3[:, 2 * jj:2 * jj + 2, :],
                        start=(jj == 0), stop=(jj == NB // 2 - 1),
                        perf_mode=mybir.MatmulPerfMode.DoubleRow)
                if pending is not None:
                    pending()
                pending = sink(i, pt)
            pending()

        def dconv():
            for i in range(NB):
                xst_transposes((False, 0), i, x0sb[:, i * SC:(i + 1) * SC])
            for s, s8 in ((0, sa8), (1, sb8)):

                def x1_sink(i, pt, s=s):
                    xb = x1q3[:, i]
                    nc.scalar.mul(xb, pt[:], 1.0 / S_SCALE)

                    def deferred():
                        xst_transposes((True, s), i,
                                       x1q[:, i * SC:(i + 1) * SC])
                    return deferred

                spmm(s8, x0q, x1_sink)

                def x2_sink(i, pt, s=s):
                    blk = x2p.tile([128, SC], BF16, tag="x2")
                    nc.vector.scalar_tensor_tensor(
                        blk[:], x0sb[:, i * SC:(i + 1) * SC],
                        -(PROD / 2.0), pt[:],
                        mybir.AluOpType.mult, mybir.AluOpType.add)

                    def deferred():
                        xst_transposes((False, 1 + s), i, blk)
                    return deferred

                spmm(s8, x1q, x2_sink)

        def proj_mms(b, c, gate):
            """Projection psum for (b, n-chunk c): host-input preload +
            3 bf16 K=64 matmuls + 1 fp8 DoubleRow (m1+m3)."""
            ws, w83, pind, O = ((wgs, wg83, pgid, 128) if gate
                                else (wcs, wc83, pcid, U))
            pin = pinp.tile([O, 512], F32, tag="pin")
            nc.sync.dma_start(
                pin[:], pind[b * O:(b + 1) * O, c * 512:(c + 1) * 512])
            pt = psum.tile([O, 512], F32, tag="ps", name="po")
            bp = (b % 2) * U
            for g in range(3):
                rs = xst_s(g, b // 2)[bp:bp + U, c * 512:(c + 1) * 512]
                nc.tensor.matmul(pt[:], ws[bp:bp + U, g * O:(g + 1) * O],
                                 rs, start=(g == 0), stop=False)
            r8 = xst83[bp:bp + U, :, b // 2, c * 512:(c + 1) * 512]
            nc.tensor.matmul(pt[:], w83[bp:bp + U, :, :O], r8,
                             start=False, stop=True,
                             perf_mode=mybir.MatmulPerfMode.DoubleRow)
            # add the host-computed input-feature contribution
            nc.vector.scalar_tensor_tensor(
                pt[:], pin[:], 1.0, pt[:],
                mybir.AluOpType.mult, mybir.AluOpType.add)
            return pt

        SIG = mybir.ActivationFunctionType.Sigmoid

        def w_stage_gate():
            """sigmoid -> r (x0sb *= r^T in place), u -> DRAM (bf16)."""
            ubv = ubufd.rearrange("p (i b u) -> p i b u", b=BL, u=U)
            pending = None
            for b in range(BL):
                for c in range(4):
                    pt = proj_mms(b, c, True)
                    rb = small.tile([U, 512], BF16, tag="rb")
                    nc.scalar.activation(rb[:], pt[:U, :], SIG,
                                         bias=bg[:U, :], scale=1.0 / W_SCALE)
                    ub = small.tile([U, 512], BF16, tag="ub")
                    nc.scalar.activation(ub[:], pt[U:128, :], SIG,
                                         bias=bg[U:128, :],
                                         scale=1.0 / W_SCALE)

                    def deferred(b=b, c=c, rb=rb, ub=ub):
                        rpt = psum.tile([128, 256], BF16, tag="ps")
                        upt = psum.tile([128, 256], BF16, tag="ps")
                        for j in range(4):
                            nc.tensor.transpose(
                                rpt[:, j * U:(j + 1) * U],
                                rb[:, j * 128:(j + 1) * 128], identb[:U, :U])
                            nc.tensor.transpose(
                                upt[:, j * U:(j + 1) * U],
                                ub[:, j * 128:(j + 1) * 128], identb[:U, :U])
                        xv = x0sb[:].rearrange("p (i c) -> p i c", c=SC)[
                            :, 4 * c:4 * c + 4, b * U:(b + 1) * U]
                        nc.vector.tensor_mul(
                            xv, xv,
                            rpt[:].rearrange("p (i o) -> p i o", o=U))
                        ubb = ubp.tile([128, 256], BF16, tag="ubb")
                        nc.vector.tensor_copy(ubb[:], upt[:])
                        nc.sync.dma_start(
                            ubv[:, 4 * c:4 * c + 4, b, :],
                            ubb[:].rearrange("p (i o) -> p i o", o=U))
                    if pending is not None:
                        pending()
                    pending = deferred
            pending()

        def w_stage_cand():
            """tanh -> c; fused GRU combine per (b, n-half)."""
            ubv = ubufd.rearrange("p (i b u) -> p i b u", b=BL, u=U)
            pending = None
            for b in range(BL):
                for h in range(2):
                    stt = stp.tile([128, 512], BF16, tag="stt")
                    nc.sync.dma_start(
                        stt[:].rearrange("p (i u) -> p i u", u=U),
                        stfv[:, 8 * h:8 * h + 8, b * U:(b + 1) * U])
                    ut = utp.tile([128, 512], BF16, tag="ut")
                    nc.sync.dma_start(
                        ut[:].rearrange("p (i u) -> p i u", u=U),
                        ubv[:, 8 * h:8 * h + 8, b, :])
                    cbs = []
                    for cc in range(2):
                        pt = proj_mms(b, 2 * h + cc, False)
                        cb = cbp.tile([U, 512], BF16, tag="cb")
                        nc.scalar.activation(
                            cb[:], pt[:, :],
                            mybir.ActivationFunctionType.Tanh, bias=bc[:],
                            scale=1.0 / W_SCALE)
                        cbs.append(cb)

                    def deferred(b=b, h=h, cbs=cbs, stt=stt, ut=ut):
                        cpt = psum.tile([128, 512], BF16, tag="ps")
                        for cc in range(2):
                            for j in range(4):
                                nc.tensor.transpose(
                                    cpt[:, cc * 256 + j * U:
                                        cc * 256 + (j + 1) * U],
                                    cbs[cc][:, j * 128:(j + 1) * 128],
                                    identb[:U, :U])
                        tf = tfp.tile([128, 512], F32, tag="tf")
                        # tf = (state - c) * u + c
                        nc.vector.tensor_sub(tf[:], stt[:], cpt[:])
                        nc.vector.tensor_mul(tf[:], tf[:], ut[:])
                        nc.vector.tensor_add(tf[:], tf[:], cpt[:])
                        nc.sync.dma_start(
                            outv[:, 8 * h:8 * h + 8, b * U:(b + 1) * U],
                            tf[:].rearrange("p (i u) -> p i u", u=U))
                    if pending is not None:
                        pending()
                    pending = deferred
            pending()

        # ---- gate ----
        dconv()
        w_stage_gate()
        # ---- candidate (x0sb now holds candX) ----
        nc.scalar.mul(x0q[:], x0sb[:], X_SCALE)
        dconv()
        w_stage_cand()


_NC_CACHE = {}


def _get_nc():
    if "nc" not in _NC_CACHE:
        _NC_CACHE["nc"] = _build_nc()
    return _NC_CACHE["nc"]


def _host_prep(inputs, state, edges1, vals1, edges2, vals2, W_gate, b_gate,
               W_cand, b_cand):
    import ml_dtypes
    BF = ml_dtypes.bfloat16
    E4 = ml_dtypes.float8_e4m3
    inputs = np.asarray(inputs, np.float32)
    state = np.asarray(state, np.float32)
    Wg = np.asarray(W_gate, np.float32).reshape(F, M, 2 * U)
    Wc = np.asarray(W_cand, np.float32).reshape(F, M, U)

    def densify(edges, vals, transpose):
        S = np.zeros((N, N), np.float32)
        r, c = (1, 0) if transpose else (0, 1)
        np.add.at(S, (np.asarray(edges[r]).astype(np.int64),
                      np.asarray(edges[c]).astype(np.int64)),
                  np.asarray(vals, np.float32))
        return S

    SaT = densify(edges1, vals1, True)
    SbT = densify(edges2, vals2, True)
    SaT8 = (SaT * S_SCALE).astype(E4)
    SbT8 = (SbT * S_SCALE).astype(E4)

    def reorder(Wm):
        O = Wm.shape[2]
        # bf16 groups m0/m2/m4 (x W_SCALE; m2/m4 also 2/PROD for the
        # 8192*x2' storage scale), duplicated at partition bases 0/64
        Ws = np.empty((U, 3, O), np.float32)
        Ws[:, 0] = Wm[D_IN:, 0] * W_SCALE
        Ws[:, 1] = Wm[D_IN:, 2] * (W_SCALE * 2.0 / PROD)
        Ws[:, 2] = Wm[D_IN:, 4] * (W_SCALE * 2.0 / PROD)
        Ws = Ws.reshape(U, 3 * O)
        Ws2 = np.concatenate([Ws, Ws], 0)
        # fp8 pair m1/m3: 16*W against xs values 16*x1 -> 256 = W_SCALE
        W8 = np.empty((U, 2, O), np.float32)
        W8[:, 0] = Wm[D_IN:, 1] * (W_SCALE / X_SCALE)
        W8[:, 1] = Wm[D_IN:, 3] * (W_SCALE / X_SCALE)
        W8 = W8.reshape(U, 2 * O)
        W82 = np.concatenate([W8, W8], 0)
        return Ws2.astype(BF), W82.astype(E4)

    wgs, wg8 = reorder(Wg)
    wcs, wc8 = reorder(Wc)
    bgv = np.asarray(b_gate, np.float32).reshape(128, 1)
    bcv = np.asarray(b_cand, np.float32).reshape(U, 1)

    # host-side input-feature diffusion + projection (exact f32)
    Sa = densify(edges1, vals1, False)
    Sb = densify(edges2, vals2, False)
    Z = np.ascontiguousarray(
        inputs.reshape(B, N, D_IN).transpose(1, 0, 2).reshape(N, B * D_IN))
    z1a = Sa @ Z
    z2a = 2.0 * (Sa @ z1a) - Z
    z1b = Sb @ Z
    z2b = 2.0 * (Sb @ z1b) - Z
    xs_in = np.stack([Z, z1a, z2a, z1b, z2b], 0).reshape(M, N, B, D_IN)
    # pg[b, o, n] = sum_{m, fi} xs_in[m, n, b, fi] * W[fi, m, o]
    pg = np.einsum('mnbf,fmo->bon', xs_in, Wg[:D_IN] * W_SCALE,
                   optimize=True).astype(np.float32)
    pc = np.einsum('mnbf,fmo->bon', xs_in, Wc[:D_IN] * W_SCALE,
                   optimize=True).astype(np.float32)

    in_maps = []
    for cid in range(NCORES):
        bsl = slice(cid * BL, (cid + 1) * BL)
        st_c = state[bsl].reshape(BL, N, U)
        x0 = np.ascontiguousarray(st_c.transpose(1, 0, 2).reshape(N, SC))
        in_maps.append(dict(
            x0=x0.astype(BF),
            x0q=(x0 * X_SCALE).astype(E4),
            sat8=SaT8, sbt8=SbT8, wgs=wgs, wg8=wg8, wcs=wcs, wc8=wc8,
            bg=bgv, bc=bcv,
            pgi=np.ascontiguousarray(pg[bsl].reshape(BL * 128, N)),
            pci=np.ascontiguousarray(pc[bsl].reshape(BL * U, N)),
        ))
    return in_maps


def kernel(**inputs):
    nc = _get_nc()
    in_maps = _host_prep(**inputs)
    res = run_bass_kernel_spmd(nc, in_maps, list(range(NCORES)))
    outs = []
    for c in range(NCORES):
        o = np.asarray(res.results[c]["out"])          # [N, (b, u)]
        outs.append(o.reshape(N, BL, U).transpose(1, 0, 2).reshape(BL, N * U))
    return np.concatenate(outs, 0).astype(np.float32)
